# revision 17
# baseline (speedup 1.0000x reference)
"""Causal multi-head attention on 8 Trainium2 NeuronCores.

Problem: B=2, S=4096, D_MODEL=768, H=12, D_HEAD=64, fp32 I/O.

Sharding: (batch, head-group) -> core.  Cores 0-3 take batch 0, cores 4-7
take batch 1; each core computes 3 of the 12 heads for its batch and emits a
partial output [S, D_MODEL] (its heads' contribution to the W_O contraction).
The host sums the 4 partials per batch and adds b_O.

Per-core device program (matmul compute in bf16, fp32 PSUM accumulation):
  1. QT/KT[z, t] = W.T @ xT; heads 0,1 packed on partition halves (0-63 /
     64-127) so their scores matmuls run concurrently in different PE row
     groups; head 2 separate.  W_Q is pre-scaled by 1/8 host-side.  VT
     computed per head-pair/single, then PE-transposed to V[t, z] with a ones
     column appended (softmax row sums ride along the AV matmul).
  2. Flash attention over 512-wide query windows, one interleaved loop per
     window covering all three heads per k-tile.  The three exps are split
     across engines so no engine gates the PE: head 0 on ScalarE (ACT Exp),
     heads 1,2 on VectorE via a fused Schraudolph fast-exp (tensor_scalar
     mult+add with int16 output whose bits ARE the bf16 encoding of e^x;
     ~2% rel err, fine at the 2e-2 gate).  Causal diag masks run on GpSimd.
     Score matmuls are emitted one k-tile ahead of the AV matmuls so the PE
     never waits on an exp.  PSUM: 4-slot score ring + 3 z slots + 1 fill
     slot = 8 banks exactly.
  3. Softmax normalization: row sums (row 64 of each z psum) are copied out,
     inverted with the 1-op approx reciprocal, broadcast across partitions by
     a tiny rank-2/rank-1 f32r matmul, and applied with one multiply per
     head-pair.  z for heads 0,1 is restacked onto partitions 0:63 / 64:127
     (SBUF->SBUF DMA for the high half) so the output projection contracts
     both heads in a single C=128 matmul.
  4. Output projection per 128-row tile: one C=128 matmul (heads 0,1) plus
     one C=64 accumulating matmul (head 2) per 384-wide half; psum is copied
     out on ScalarE/VectorE alternately and DMA'd.  Normalization broadcasts
     and O-proj tiles are deferred into a fill list drained one-two per
     k-tile to keep the PE dense (a >3.4us PE stall re-throttles the PE
     clock from 2.4 to 1.2 GHz).
"""

import numpy as np
import ml_dtypes

B, S, DM, H, DH = 2, 4096, 768, 12, 64
NCORES = 8
GROUPS = 4                  # head-groups per batch
HPC = H // GROUPS           # heads per core = 3
P = 128
QCH = 512                   # psum bank width (fp32)

_BF = ml_dtypes.bfloat16

# Schraudolph fast-exp constants: int16 value = round(x*AEXP + BEXP) is the
# bf16 bit pattern of e^x (C=367400 tuned for min max-rel-err, ~2% RMS).
_AEXP = (2.0 ** 23 / np.log(2.0)) / 65536.0
_BEXP = (127.0 * 2.0 ** 23 - 367400.0) / 65536.0

_cache = {}


def _build(seq_len, use_biases):
    import concourse.bacc as bacc
    import concourse.mybir as mybir
    import concourse.tile as tile

    f32 = mybir.dt.float32
    f32r = mybir.dt.float32r
    bf16 = mybir.dt.bfloat16
    i16 = mybir.dt.int16
    Exp = mybir.ActivationFunctionType.Exp
    mult = mybir.AluOpType.mult
    add = mybir.AluOpType.add

    SQ = seq_len
    n_kt = SQ // P               # k tiles
    n_ch = SQ // QCH             # 512-wide chunks
    DSL = DM // P                # contraction slices for the projections
    KPW = QCH // P               # k tiles per query window

    nc = bacc.Bacc(None, target_bir_lowering=False)

    xT = nc.declare_dram_parameter("xT", [DM, SQ], bf16, isOutput=False)
    wq = nc.declare_dram_parameter("wq", [DM, HPC * DH], bf16, isOutput=False)
    wk = nc.declare_dram_parameter("wk", [DM, HPC * DH], bf16, isOutput=False)
    wv = nc.declare_dram_parameter("wv", [DM, HPC * DH], bf16, isOutput=False)
    wo_p = nc.declare_dram_parameter("wo_p", [2 * DH, DM], bf16, isOutput=False)
    wo_s = nc.declare_dram_parameter("wo_s", [DH, DM], bf16, isOutput=False)
    trimask = nc.declare_dram_parameter("trimask", [P, P], bf16, isOutput=False)
    ident_b = nc.declare_dram_parameter("ident_b", [P, P], bf16, isOutput=False)
    ones_z = nc.declare_dram_parameter("ones_z", [1, DH], bf16, isOutput=False)
    e_lo = nc.declare_dram_parameter("e_lo", [1, P], bf16, isOutput=False)
    e_hi = nc.declare_dram_parameter("e_hi", [1, P], bf16, isOutput=False)
    if use_biases:
        bqkv_p = nc.declare_dram_parameter("bqkv_p", [P, 3], f32, isOutput=False)
        bqkv_s = nc.declare_dram_parameter("bqkv_s", [DH, 3], f32, isOutput=False)
    out = nc.declare_dram_parameter("out", [SQ, DM], f32, isOutput=True)

    with tile.TileContext(nc) as tc:
        with (
            tc.tile_pool(name="singles", bufs=1) as singles,
            tc.tile_pool(name="persist", bufs=1) as persist,
            tc.tile_pool(name="nrm", bufs=2) as nrm,
        ):
            # ---- constants / weights ----
            w_sb = {}
            for name, drm in (("q", wq), ("k", wk), ("v", wv)):
                t = singles.tile([P, DSL, HPC * DH], bf16, tag=f"w{name}")
                nc.sync.dma_start(t[:], drm.rearrange("(o p) c -> p o c", p=P))
                w_sb[name] = t
            wop_sb = singles.tile([2 * DH, DM], bf16)
            nc.sync.dma_start(wop_sb[:], wo_p[:])
            wos_sb = singles.tile([DH, DM], bf16)
            nc.sync.dma_start(wos_sb[:], wo_s[:])
            tri_sb = singles.tile([P, P], bf16)
            nc.sync.dma_start(tri_sb[:], trimask[:])
            idb_sb = singles.tile([P, P], bf16)
            nc.sync.dma_start(idb_sb[:], ident_b[:])
            ones_sb = singles.tile([1, DH], bf16)
            nc.sync.dma_start(ones_sb[:], ones_z[:])
            elo_sb = singles.tile([1, P], bf16, tag="elo")
            nc.sync.dma_start(elo_sb[:], e_lo[:])
            ehi_sb = singles.tile([1, P], bf16, tag="ehi")
            nc.sync.dma_start(ehi_sb[:], e_hi[:])
            bias_p = bias_s = None
            if use_biases:
                bias_p = singles.tile([P, 3], f32, tag="bp")
                nc.sync.dma_start(bias_p[:], bqkv_p[:])
                bias_s = singles.tile([DH, 3], f32, tag="bs")
                nc.sync.dma_start(bias_s[:], bqkv_s[:])

            # ---- persistent activations ----
            QT2 = persist.tile([P, SQ], bf16, tag="QT2")   # heads 0,1 stacked
            KT2 = persist.tile([P, SQ], bf16, tag="KT2")
            QTs = persist.tile([DH, SQ], bf16, tag="QTs")  # head 2
            KTs = persist.tile([DH, SQ], bf16, tag="KTs")
            V_sb = persist.tile([P, HPC, n_kt, DH + 1], bf16, tag="V")
            Zn2 = persist.tile([P, SQ], bf16, tag="Zn2")   # normalized z h0|h1
            Zns = persist.tile([DH, SQ], bf16, tag="Zns")  # normalized z h2

            # ================= QKV projections =================
            with (
                tc.tile_pool(name="xT_pool", bufs=1) as xT_pool,
                tc.tile_pool(name="qkv_ps", bufs=3, space="PSUM") as qkv_ps,
                tc.tile_pool(name="vt_ps", bufs=3, space="PSUM") as vt_ps,
                tc.tile_pool(name="vt_sb", bufs=1) as vt_pool,
            ):
                # one chunked DMA per 512-col slab: single queue-issue each,
                # fine-grained enough that proj chunk c starts early.
                xT_sb = xT_pool.tile([P, DSL, SQ], bf16)
                xT_r = xT.rearrange("(o p) c -> p o c", p=P)
                for c in range(n_ch):
                    nc.sync.dma_start(
                        xT_sb[:, :, c * QCH:(c + 1) * QCH],
                        xT_r[:, :, c * QCH:(c + 1) * QCH])
                # HAM warm-up: keep the PE MAC-busy while xT streams in, so
                # the first real matmuls run at 2.4 GHz instead of 1.2.
                wup = vt_ps.tile([P, P], f32, tag="vtp", name="wup")
                for _ in range(40):
                    nc.tensor.matmul(wup[:], lhsT=idb_sb[:], rhs=idb_sb[:],
                                     start=True, stop=True)

                def proj(tname, w_cols, dst, bias, c, evac):
                    m = w_cols.stop - w_cols.start
                    ps = qkv_ps.tile([P, QCH], f32, tag="proj",
                                     name="proj_ps")[:m]
                    for o in range(DSL):
                        nc.tensor.matmul(
                            ps[:],
                            lhsT=w_sb[tname][:, o, w_cols],
                            rhs=xT_sb[:, o, c * QCH:(c + 1) * QCH],
                            start=(o == 0), stop=(o == DSL - 1),
                        )
                    if bias is not None:
                        if evac == 0:
                            nc.scalar.add(dst, ps[:], bias)
                        else:
                            nc.vector.tensor_scalar_add(dst, ps[:], bias)
                    else:
                        if evac == 0:
                            nc.scalar.copy(dst, ps[:])
                        else:
                            nc.vector.tensor_copy(dst, ps[:])

                for tname, d2, ds, bi in (("q", QT2, QTs, 0),
                                          ("k", KT2, KTs, 1)):
                    for c in range(n_ch):
                        proj(tname, slice(0, P),
                             d2[:, c * QCH:(c + 1) * QCH],
                             bias_p[:, bi:bi + 1] if use_biases else None,
                             c, c % 2)
                    for c in range(n_ch):
                        proj(tname, slice(P, P + DH),
                             ds[:, c * QCH:(c + 1) * QCH],
                             bias_s[:, bi:bi + 1] if use_biases else None,
                             c, c % 2)
                # V: pair pass (M=128) + single pass, then batched transposes
                vt2 = vt_pool.tile([P, SQ], bf16, tag="vt2")
                for c in range(n_ch):
                    proj("v", slice(0, P), vt2[:, c * QCH:(c + 1) * QCH],
                         bias_p[:, 2:3] if use_biases else None, c, c % 2)
                vts = vt_pool.tile([DH, SQ], bf16, tag="vts")
                for c in range(n_ch):
                    proj("v", slice(P, P + DH),
                         vts[:, c * QCH:(c + 1) * QCH],
                         bias_s[:, 2:3] if use_biases else None, c, c % 2)
                # paired transposes: 2 k-tiles per psum tile / per DVE copy
                for kt in range(0, n_kt, 2):
                    for h, srcs, idsl in (
                            (0, vt2[0:DH], idb_sb[:DH, :DH]),
                            (1, vt2[DH:P], idb_sb[DH:P, DH:P]),
                            (2, vts[:], idb_sb[:DH, :DH])):
                        vp = vt_ps.tile([P, 2 * DH], bf16, tag="vtp",
                                        name="vp")
                        nc.tensor.transpose(
                            vp[:, 0:DH], srcs[:, kt * P:(kt + 1) * P], idsl)
                        nc.tensor.transpose(
                            vp[:, DH:2 * DH],
                            srcs[:, (kt + 1) * P:(kt + 2) * P], idsl)
                        nc.vector.tensor_copy(
                            V_sb[:, h, kt:kt + 2, 0:DH],
                            vp[:].rearrange("p (j z) -> p j z", j=2))
                nc.vector.memset(V_sb[:, :, :, DH:DH + 1], 1.0)

            # ===== flash: interleaved heads, one 512-wide window loop ====
            with (
                tc.tile_pool(name="sc_ps", bufs=4, space="PSUM") as sc_ps,
                tc.tile_pool(name="z_ps", bufs=3, space="PSUM") as z_ps,
                tc.tile_pool(name="fill_ps", bufs=1, space="PSUM") as fill_ps,
                tc.tile_pool(name="pt_sb", bufs=3) as pt_pool,
                tc.tile_pool(name="o_sb", bufs=6) as o_pool,
            ):
                HD = DM // 2
                fills = []

                def pop_fills(k=2):
                    for _ in range(k):
                        if fills:
                            fills.pop(0)()

                def stage2_thunks(qs, st):
                    """Broadcast row sums, invert post-broadcast, normalize."""
                    rra, rrb_, rrc, zsb2, zsbc = st
                    q0 = qs * QCH

                    def th_pair():
                        rb = fill_ps.tile([P, QCH], f32, tag="fill",
                                          name="rb2")
                        nc.tensor.matmul(rb[:], lhsT=elo_sb[:], rhs=rra[:],
                                         start=True, stop=False)
                        nc.tensor.matmul(rb[:], lhsT=ehi_sb[:], rhs=rrb_[:],
                                         start=False, stop=True)
                        rq = nrm.tile([P, QCH], f32, tag="rq2", name="rq2")
                        nc.vector.reciprocal_approx_fast(rq[:], rb[:])
                        nc.vector.tensor_tensor(
                            Zn2[:, q0:q0 + QCH], zsb2[:], rq[:], mult)

                    def th_single():
                        rb = fill_ps.tile([P, QCH], f32, tag="fill",
                                          name="rbc")[:DH]
                        nc.tensor.matmul(rb[:], lhsT=ones_sb[:], rhs=rrc[:],
                                         start=True, stop=True)
                        rq = nrm.tile([DH, QCH], f32, tag="rqs", name="rqs")
                        nc.vector.reciprocal_approx_fast(rq[:], rb[:])
                        nc.vector.tensor_tensor(
                            Zns[:, q0:q0 + QCH], zsbc[:], rq[:], mult)

                    return [th_pair, th_single]

                def oproj_thunks(w):
                    """O-proj for window w as per-half-tile thunks."""
                    thunks = []
                    for tt in range(w * KPW, (w + 1) * KPW):
                        osb = o_pool.tile([P, DM], f32, tag="osb", name="osb")

                        def th(tt=tt, osb=osb, half=0):
                            po = fill_ps.tile([P, QCH], f32, tag="fill",
                                              name="po")[:, :HD]
                            nc.tensor.matmul(
                                po[:],
                                lhsT=Zn2[:, tt * P:(tt + 1) * P],
                                rhs=wop_sb[:, half * HD:(half + 1) * HD],
                                start=True, stop=False)
                            nc.tensor.matmul(
                                po[:],
                                lhsT=Zns[:, tt * P:(tt + 1) * P],
                                rhs=wos_sb[:, half * HD:(half + 1) * HD],
                                start=False, stop=True)
                            if half == 0:
                                nc.scalar.copy(
                                    osb[:, half * HD:(half + 1) * HD], po[:])
                            else:
                                nc.vector.tensor_copy(
                                    osb[:, half * HD:(half + 1) * HD], po[:])
                                nc.sync.dma_start(
                                    out[tt * P:(tt + 1) * P, :], osb[:])

                        thunks.append(th)
                        thunks.append(lambda tt=tt, osb=osb, th=th: th(tt, osb, 1))
                    return thunks

                for qs in range(n_ch):
                    q0 = qs * QCH
                    nk = KPW * qs + KPW
                    za = z_ps.tile([DH + 1, QCH], f32, tag="z", name="za")
                    zb = z_ps.tile([DH + 1, QCH], f32, tag="z", name="zb")
                    zc = z_ps.tile([DH + 1, QCH], f32, tag="z", name="zc")
                    pend = None
                    for ki in range(nk):
                        vs = max(0, P * ki - q0)
                        sa = sc_ps.tile([P, QCH], f32, tag="S", name="sa")
                        sb = sc_ps.tile([P, QCH], f32, tag="S", name="sb")
                        sc = sc_ps.tile([P, QCH], f32, tag="S", name="sc")
                        nc.tensor.matmul(
                            sa[:, vs:], lhsT=KT2[0:DH, ki * P:(ki + 1) * P],
                            rhs=QT2[0:DH, q0 + vs:q0 + QCH],
                            start=True, stop=True)
                        nc.tensor.matmul(
                            sb[:, vs:], lhsT=KT2[DH:P, ki * P:(ki + 1) * P],
                            rhs=QT2[DH:P, q0 + vs:q0 + QCH],
                            start=True, stop=True)
                        nc.tensor.matmul(
                            sc[:, vs:], lhsT=KTs[:, ki * P:(ki + 1) * P],
                            rhs=QTs[:, q0 + vs:q0 + QCH],
                            start=True, stop=True)
                        pta = pt_pool.tile([P, QCH], bf16, tag="pta",
                                           name="pta")
                        nc.scalar.activation(pta[:, vs:], sa[:, vs:], Exp)
                        ptb = pt_pool.tile([P, QCH], i16, tag="ptb",
                                           name="ptb")
                        nc.vector.tensor_scalar(
                            ptb[:, vs:], sb[:, vs:], _AEXP, _BEXP, mult, add)
                        ptc = pt_pool.tile([P, QCH], i16, tag="ptc",
                                           name="ptc")
                        nc.vector.tensor_scalar(
                            ptc[:, vs:], sc[:, vs:], _AEXP, _BEXP, mult, add)
                        ptb_bf = ptb[:].bitcast(bf16)
                        ptc_bf = ptc[:].bitcast(bf16)
                        if ki >= KPW * qs:  # diagonal tile: causal mask
                            for blk in (pta[:, vs:vs + P],
                                        ptb_bf[:, vs:vs + P],
                                        ptc_bf[:, vs:vs + P]):
                                nc.gpsimd.tensor_tensor(
                                    blk, blk, tri_sb[:], mult)
                        if ki == 0:
                            def emit_avs(kj, vj, a, b_, c_,
                                         za=za, zb=zb, zc=zc, nk=nk):
                                for h, zt, pt_ in ((0, za, a), (1, zb, b_),
                                                   (2, zc, c_)):
                                    nc.tensor.matmul(
                                        zt[:, vj:], lhsT=V_sb[:, h, kj, :],
                                        rhs=pt_[:, vj:],
                                        start=(kj == 0), stop=(kj == nk - 1))
                        if pend is not None:
                            emit_avs(*pend)
                            pop_fills()
                        pend = (ki, vs, pta, ptb_bf, ptc_bf)
                    emit_avs(*pend)
                    pop_fills()

                    # ---- stage1 inline: extract row sums; evacuate z ----
                    rro = []
                    for zt, nm in ((za, "a"), (zb, "b"), (zc, "c")):
                        rr = nrm.tile([1, QCH], bf16, tag=f"rr{nm}",
                                      name=f"rr{nm}")
                        nc.vector.tensor_copy(rr[:], zt[DH:DH + 1, :])
                        rro.append(rr)
                    rra, rrb_, rrc = rro
                    zsb2 = nrm.tile([P, QCH], bf16, tag="zsb2", name="zsb2")
                    nc.vector.tensor_copy(zsb2[0:DH, :], za[0:DH, :])
                    tmpb = nrm.tile([DH, QCH], bf16, tag="tmpb", name="tmpb")
                    nc.vector.tensor_copy(tmpb[:], zb[0:DH, :])
                    nc.sync.dma_start(zsb2[DH:P, :], tmpb[:])
                    zsbc = nrm.tile([DH, QCH], bf16, tag="zsbc", name="zsbc")
                    nc.vector.tensor_copy(zsbc[:], zc[0:DH, :])

                    # ---- stage the deferred work as PE fillers ----
                    fills.extend(
                        stage2_thunks(qs, (rra, rrb_, rrc, zsb2, zsbc)))
                    if qs >= 1:
                        fills.extend(oproj_thunks(qs - 1))
                while fills:
                    fills.pop(0)()
                for th in oproj_thunks(n_ch - 1):
                    th()

    nc.compile()
    return nc


def _prep_inputs(inputs, seq_len, use_biases):
    x = np.asarray(inputs["normalized_resid_pre"], dtype=np.float32)
    WQ = np.asarray(inputs["W_Q"], dtype=np.float32)
    WK = np.asarray(inputs["W_K"], dtype=np.float32)
    WV = np.asarray(inputs["W_V"], dtype=np.float32)
    WO = np.asarray(inputs["W_O"], dtype=np.float32)

    tri = np.triu(np.ones((P, P), np.float32)).astype(_BF)  # keep j >= p
    idb = np.eye(P, dtype=np.float32).astype(_BF)
    onz = np.ones((1, DH), np.float32).astype(_BF)
    elo = np.zeros((1, P), np.float32)
    elo[0, :DH] = 1.0
    elo = elo.astype(_BF)
    ehi = np.zeros((1, P), np.float32)
    ehi[0, DH:] = 1.0
    ehi = ehi.astype(_BF)

    in_maps = []
    for c in range(NCORES):
        b, g = divmod(c, GROUPS)
        hs = slice(g * HPC, (g + 1) * HPC)
        wo_g = WO[hs]  # [3, 64, 768]
        m = {
            "xT": np.ascontiguousarray(x[b, :seq_len].T).astype(_BF),
            # W_Q pre-scaled by 1/sqrt(d_head)=1/8 so scores psum = s/8
            "wq": np.ascontiguousarray(
                (WQ[hs] / 8.0).transpose(1, 0, 2).reshape(DM, HPC * DH)
            ).astype(_BF),
            "wk": np.ascontiguousarray(
                WK[hs].transpose(1, 0, 2).reshape(DM, HPC * DH)).astype(_BF),
            "wv": np.ascontiguousarray(
                WV[hs].transpose(1, 0, 2).reshape(DM, HPC * DH)).astype(_BF),
            "wo_p": np.ascontiguousarray(
                wo_g[0:2].reshape(2 * DH, DM)).astype(_BF),
            "wo_s": np.ascontiguousarray(wo_g[2]).astype(_BF),
            "trimask": tri,
            "ident_b": idb,
            "ones_z": onz,
            "e_lo": elo,
            "e_hi": ehi,
        }
        if use_biases:
            bq = np.asarray(inputs["b_Q"], np.float32)[hs] / 8.0
            bk = np.asarray(inputs["b_K"], np.float32)[hs]
            bv = np.asarray(inputs["b_V"], np.float32)[hs]
            # pair layout: [128, 3] = heads {0,1} stacked, cols q/k/v
            m["bqkv_p"] = np.stack(
                [np.concatenate([bq[0], bq[1]]),
                 np.concatenate([bk[0], bk[1]]),
                 np.concatenate([bv[0], bv[1]])], axis=1)
            m["bqkv_s"] = np.stack([bq[2], bk[2], bv[2]], axis=1)
        in_maps.append(m)
    return in_maps


TRACE = False          # test.py can flip this to get exec_time_ns
last_result = None     # BassKernelResults of the most recent run


def kernel(seq_len=S, **inputs):
    global last_result
    from concourse.bass_utils import run_bass_kernel_spmd

    use_biases = any(
        np.any(np.asarray(inputs[k]) != 0) for k in ("b_Q", "b_K", "b_V"))

    key = (seq_len, use_biases)
    if key not in _cache:
        _cache[key] = _build(seq_len, use_biases)
    nc = _cache[key]

    in_maps = _prep_inputs(inputs, seq_len, use_biases)
    res = run_bass_kernel_spmd(nc, in_maps, core_ids=list(range(NCORES)),
                               trace=TRACE)
    last_result = res

    b_O = np.asarray(inputs["b_O"], dtype=np.float32)
    out = np.zeros((B, seq_len, DM), np.float32)
    for c in range(NCORES):
        b = c // GROUPS
        out[b] += np.asarray(res.results[c]["out"], dtype=np.float32)
    out += b_O[None, None, :]
    return out


# revision 20
# speedup vs baseline: 1.1759x; 1.1759x over previous
"""Causal multi-head attention on 8 Trainium2 NeuronCores.

Problem: B=2, S=4096, D_MODEL=768, H=12, D_HEAD=64, fp32 I/O.

Sharding: (batch, head-group) -> core.  Cores 0-3 take batch 0, cores 4-7
take batch 1; each core computes 3 of the 12 heads for its batch and emits a
partial output [S, D_MODEL] (its heads' contribution to the W_O contraction).
The host sums the 4 partials per batch and adds b_O.

Per-core device program (matmul compute in bf16, fp32 PSUM accumulation):
  1. QT/KT[z, t] = W.T @ xT; heads 0,1 packed on partition halves (0-63 /
     64-127) so their scores matmuls run concurrently in different PE row
     groups; head 2 separate.  W_Q is pre-scaled by 1/8 host-side.  VT
     computed per head-pair/single, then PE-transposed to V[t, z] with a ones
     column appended (softmax row sums ride along the AV matmul).
  2. Flash attention over 512-wide query windows, one interleaved loop per
     window covering all three heads per k-tile.  The three exps are split
     across engines so no engine gates the PE: head 0 on ScalarE (ACT Exp),
     heads 1,2 on VectorE via a fused Schraudolph fast-exp (tensor_scalar
     mult+add with int16 output whose bits ARE the bf16 encoding of e^x;
     ~2% rel err, fine at the 2e-2 gate).  Causal diag masks run on GpSimd.
     Score matmuls are emitted one k-tile ahead of the AV matmuls so the PE
     never waits on an exp.  PSUM: 4-slot score ring + 3 z slots + 1 fill
     slot = 8 banks exactly.
  3. Softmax normalization: row sums (row 64 of each z psum) are copied out,
     inverted with the 1-op approx reciprocal, broadcast across partitions by
     a tiny rank-2/rank-1 f32r matmul, and applied with one multiply per
     head-pair.  z for heads 0,1 is restacked onto partitions 0:63 / 64:127
     (SBUF->SBUF DMA for the high half) so the output projection contracts
     both heads in a single C=128 matmul.
  4. Output projection per 128-row tile: one C=128 matmul (heads 0,1) plus
     one C=64 accumulating matmul (head 2) per 384-wide half; psum is copied
     out on ScalarE/VectorE alternately and DMA'd.  Normalization broadcasts
     and O-proj tiles are deferred into a fill list drained one-two per
     k-tile to keep the PE dense (a >3.4us PE stall re-throttles the PE
     clock from 2.4 to 1.2 GHz).
"""

import numpy as np
import ml_dtypes

B, S, DM, H, DH = 2, 4096, 768, 12, 64
NCORES = 8
GROUPS = 4                  # head-groups per batch
HPC = H // GROUPS           # heads per core = 3
P = 128
QCH = 512                   # psum bank width (fp32)

_BF = ml_dtypes.bfloat16

# Schraudolph fast-exp constants: int16 value = round(x*AEXP + BEXP) is the
# bf16 bit pattern of e^x (C=367400 tuned for min max-rel-err, ~2% RMS).
_AEXP = (2.0 ** 23 / np.log(2.0)) / 65536.0
_BEXP = (127.0 * 2.0 ** 23 - 367400.0) / 65536.0

_cache = {}


def _build(seq_len, use_biases):
    import concourse.bacc as bacc
    import concourse.mybir as mybir
    import concourse.tile as tile

    f32 = mybir.dt.float32
    f32r = mybir.dt.float32r
    bf16 = mybir.dt.bfloat16
    i16 = mybir.dt.int16
    Exp = mybir.ActivationFunctionType.Exp
    mult = mybir.AluOpType.mult
    add = mybir.AluOpType.add

    SQ = seq_len
    n_kt = SQ // P               # k tiles
    n_ch = SQ // QCH             # 512-wide chunks
    DSL = DM // P                # contraction slices for the projections
    KPW = QCH // P               # k tiles per query window

    nc = bacc.Bacc(None, target_bir_lowering=False)

    xT = nc.declare_dram_parameter("xT", [DM, SQ], bf16, isOutput=False)
    wq = nc.declare_dram_parameter("wq", [DM, HPC * DH], bf16, isOutput=False)
    wk = nc.declare_dram_parameter("wk", [DM, HPC * DH], bf16, isOutput=False)
    wv = nc.declare_dram_parameter("wv", [DM, HPC * DH], bf16, isOutput=False)
    wo_p = nc.declare_dram_parameter("wo_p", [2 * DH, DM], bf16, isOutput=False)
    wo_s = nc.declare_dram_parameter("wo_s", [DH, DM], bf16, isOutput=False)
    trimask = nc.declare_dram_parameter("trimask", [P, P], bf16, isOutput=False)
    ident_b = nc.declare_dram_parameter("ident_b", [P, P], bf16, isOutput=False)
    ones_z = nc.declare_dram_parameter("ones_z", [1, DH], bf16, isOutput=False)
    e_lo = nc.declare_dram_parameter("e_lo", [1, P], bf16, isOutput=False)
    e_hi = nc.declare_dram_parameter("e_hi", [1, P], bf16, isOutput=False)
    if use_biases:
        bqkv_p = nc.declare_dram_parameter("bqkv_p", [P, 3], f32, isOutput=False)
        bqkv_s = nc.declare_dram_parameter("bqkv_s", [DH, 3], f32, isOutput=False)
    out = nc.declare_dram_parameter("out", [SQ, DM], f32, isOutput=True)

    with tile.TileContext(nc) as tc:
        with (
            tc.tile_pool(name="singles", bufs=1) as singles,
            tc.tile_pool(name="persist", bufs=1) as persist,
            tc.tile_pool(name="nrm", bufs=2) as nrm,
        ):
            # ---- constants / weights ----
            w_sb = {}
            for name, drm in (("q", wq), ("k", wk), ("v", wv)):
                t = singles.tile([P, DSL, HPC * DH], bf16, tag=f"w{name}")
                nc.sync.dma_start(t[:], drm.rearrange("(o p) c -> p o c", p=P))
                w_sb[name] = t
            wop_sb = singles.tile([2 * DH, DM], bf16)
            nc.sync.dma_start(wop_sb[:], wo_p[:])
            wos_sb = singles.tile([DH, DM], bf16)
            nc.sync.dma_start(wos_sb[:], wo_s[:])
            tri_sb = singles.tile([P, P], bf16)
            nc.sync.dma_start(tri_sb[:], trimask[:])
            idb_sb = singles.tile([P, P], bf16)
            nc.sync.dma_start(idb_sb[:], ident_b[:])
            ones_sb = singles.tile([1, DH], bf16)
            nc.sync.dma_start(ones_sb[:], ones_z[:])
            elo_sb = singles.tile([1, P], bf16, tag="elo")
            nc.sync.dma_start(elo_sb[:], e_lo[:])
            ehi_sb = singles.tile([1, P], bf16, tag="ehi")
            nc.sync.dma_start(ehi_sb[:], e_hi[:])
            bias_p = bias_s = None
            if use_biases:
                bias_p = singles.tile([P, 3], f32, tag="bp")
                nc.sync.dma_start(bias_p[:], bqkv_p[:])
                bias_s = singles.tile([DH, 3], f32, tag="bs")
                nc.sync.dma_start(bias_s[:], bqkv_s[:])

            # ---- persistent activations ----
            QT2 = persist.tile([P, SQ], bf16, tag="QT2")   # heads 0,1 stacked
            KT2 = persist.tile([P, SQ], bf16, tag="KT2")
            QTs = persist.tile([DH, SQ], bf16, tag="QTs")  # head 2
            KTs = persist.tile([DH, SQ], bf16, tag="KTs")
            V_sb = persist.tile([P, HPC, n_kt, DH + 1], bf16, tag="V")
            Zn2 = persist.tile([P, SQ], bf16, tag="Zn2")   # normalized z h0|h1
            Zns = persist.tile([DH, SQ], bf16, tag="Zns")  # normalized z h2

            # ================= QKV projections =================
            with (
                tc.tile_pool(name="xT_pool", bufs=1) as xT_pool,
                tc.tile_pool(name="qkv_ps", bufs=3, space="PSUM") as qkv_ps,
                tc.tile_pool(name="vt_ps", bufs=3, space="PSUM") as vt_ps,
                tc.tile_pool(name="vt_sb", bufs=1) as vt_pool,
            ):
                # one chunked DMA per 512-col slab: single queue-issue each,
                # fine-grained enough that proj chunk c starts early.
                xT_sb = xT_pool.tile([P, DSL, SQ], bf16)
                xT_r = xT.rearrange("(o p) c -> p o c", p=P)
                for c in range(n_ch):
                    nc.sync.dma_start(
                        xT_sb[:, :, c * QCH:(c + 1) * QCH],
                        xT_r[:, :, c * QCH:(c + 1) * QCH])
                # HAM warm-up: keep the PE MAC-busy while xT streams in, so
                # the first real matmuls run at 2.4 GHz instead of 1.2.
                wup = vt_ps.tile([P, P], f32, tag="vtp", name="wup")
                for _ in range(40):
                    nc.tensor.matmul(wup[:], lhsT=idb_sb[:], rhs=idb_sb[:],
                                     start=True, stop=True)

                def proj(tname, w_cols, dst, bias, c, evac):
                    m = w_cols.stop - w_cols.start
                    ps = qkv_ps.tile([P, QCH], f32, tag="proj",
                                     name="proj_ps")[:m]
                    for o in range(DSL):
                        nc.tensor.matmul(
                            ps[:],
                            lhsT=w_sb[tname][:, o, w_cols],
                            rhs=xT_sb[:, o, c * QCH:(c + 1) * QCH],
                            start=(o == 0), stop=(o == DSL - 1),
                        )
                    if bias is not None:
                        if evac == 0:
                            nc.scalar.add(dst, ps[:], bias)
                        else:
                            nc.vector.tensor_scalar_add(dst, ps[:], bias)
                    else:
                        if evac == 0:
                            nc.scalar.copy(dst, ps[:])
                        else:
                            nc.vector.tensor_copy(dst, ps[:])

                for tname, d2, ds, bi in (("q", QT2, QTs, 0),
                                          ("k", KT2, KTs, 1)):
                    for c in range(n_ch):
                        proj(tname, slice(0, P),
                             d2[:, c * QCH:(c + 1) * QCH],
                             bias_p[:, bi:bi + 1] if use_biases else None,
                             c, c % 2)
                    for c in range(n_ch):
                        proj(tname, slice(P, P + DH),
                             ds[:, c * QCH:(c + 1) * QCH],
                             bias_s[:, bi:bi + 1] if use_biases else None,
                             c, c % 2)
                # V: pair pass (M=128) + single pass, then batched transposes
                vt2 = vt_pool.tile([P, SQ], bf16, tag="vt2")
                for c in range(n_ch):
                    proj("v", slice(0, P), vt2[:, c * QCH:(c + 1) * QCH],
                         bias_p[:, 2:3] if use_biases else None, c, c % 2)
                vts = vt_pool.tile([DH, SQ], bf16, tag="vts")
                for c in range(n_ch):
                    proj("v", slice(P, P + DH),
                         vts[:, c * QCH:(c + 1) * QCH],
                         bias_s[:, 2:3] if use_biases else None, c, c % 2)
                # paired transposes: 2 k-tiles per psum tile / per DVE copy
                for kt in range(0, n_kt, 2):
                    for h, srcs, idsl in (
                            (0, vt2[0:DH], idb_sb[:DH, :DH]),
                            (1, vt2[DH:P], idb_sb[DH:P, DH:P]),
                            (2, vts[:], idb_sb[:DH, :DH])):
                        vp = vt_ps.tile([P, 2 * DH], bf16, tag="vtp",
                                        name="vp")
                        nc.tensor.transpose(
                            vp[:, 0:DH], srcs[:, kt * P:(kt + 1) * P], idsl)
                        nc.tensor.transpose(
                            vp[:, DH:2 * DH],
                            srcs[:, (kt + 1) * P:(kt + 2) * P], idsl)
                        nc.vector.tensor_copy(
                            V_sb[:, h, kt:kt + 2, 0:DH],
                            vp[:].rearrange("p (j z) -> p j z", j=2))
                nc.vector.memset(V_sb[:, :, :, DH:DH + 1], 1.0)

            # ===== flash: interleaved heads, one 512-wide window loop ====
            with (
                tc.tile_pool(name="sc_ps", bufs=4, space="PSUM") as sc_ps,
                tc.tile_pool(name="z_ps", bufs=3, space="PSUM") as z_ps,
                tc.tile_pool(name="fill_ps", bufs=1, space="PSUM") as fill_ps,
                tc.tile_pool(name="pt_sb", bufs=3) as pt_pool,
                tc.tile_pool(name="o_sb", bufs=6) as o_pool,
            ):
                HD = DM // 2
                fills = []

                def pop_fills(k=2):
                    for _ in range(k):
                        if fills:
                            fills.pop(0)()

                def stage2_thunks(qs, st):
                    """Broadcast row sums, invert post-broadcast, normalize."""
                    rra, rrb_, rrc, zsb2, zsbc = st
                    q0 = qs * QCH

                    def th_pair():
                        rb = fill_ps.tile([P, QCH], f32, tag="fill",
                                          name="rb2")
                        nc.tensor.matmul(rb[:], lhsT=elo_sb[:], rhs=rra[:],
                                         start=True, stop=False)
                        nc.tensor.matmul(rb[:], lhsT=ehi_sb[:], rhs=rrb_[:],
                                         start=False, stop=True)
                        rq = nrm.tile([P, QCH], f32, tag="rq2", name="rq2")
                        nc.vector.reciprocal_approx_fast(rq[:], rb[:])
                        nc.vector.tensor_tensor(
                            Zn2[:, q0:q0 + QCH], zsb2[:], rq[:], mult)

                    def th_single():
                        rb = fill_ps.tile([P, QCH], f32, tag="fill",
                                          name="rbc")[:DH]
                        nc.tensor.matmul(rb[:], lhsT=ones_sb[:], rhs=rrc[:],
                                         start=True, stop=True)
                        rq = nrm.tile([DH, QCH], f32, tag="rqs", name="rqs")
                        nc.vector.reciprocal_approx_fast(rq[:], rb[:])
                        nc.vector.tensor_tensor(
                            Zns[:, q0:q0 + QCH], zsbc[:], rq[:], mult)

                    return [th_pair, th_single]

                osb_rr = [0]  # 5:3 scalar/vector rotation for osb copies
                _OSB_PAT = (0, 1, 0, 0, 1, 0, 0, 1)

                def oproj_thunks(w):
                    """O-proj for window w as per-half-tile thunks."""
                    thunks = []
                    for tt in range(w * KPW, (w + 1) * KPW):
                        osb = o_pool.tile([P, DM], f32, tag="osb", name="osb")

                        def th(tt=tt, osb=osb, half=0):
                            po = fill_ps.tile([P, QCH], f32, tag="fill",
                                              name="po")[:, :HD]
                            nc.tensor.matmul(
                                po[:],
                                lhsT=Zn2[:, tt * P:(tt + 1) * P],
                                rhs=wop_sb[:, half * HD:(half + 1) * HD],
                                start=True, stop=False)
                            nc.tensor.matmul(
                                po[:],
                                lhsT=Zns[:, tt * P:(tt + 1) * P],
                                rhs=wos_sb[:, half * HD:(half + 1) * HD],
                                start=False, stop=True)
                            eng = _OSB_PAT[osb_rr[0] % 8]
                            osb_rr[0] += 1
                            if eng == 0:
                                nc.scalar.copy(
                                    osb[:, half * HD:(half + 1) * HD], po[:])
                            else:
                                nc.vector.tensor_copy(
                                    osb[:, half * HD:(half + 1) * HD], po[:])
                            if half == 1:
                                nc.sync.dma_start(
                                    out[tt * P:(tt + 1) * P, :], osb[:])

                        thunks.append(th)
                        thunks.append(lambda tt=tt, osb=osb, th=th: th(tt, osb, 1))
                    return thunks

                for qs in range(n_ch):
                    q0 = qs * QCH
                    nk = KPW * qs + KPW
                    za = z_ps.tile([DH + 1, QCH], f32, tag="z", name="za")
                    zb = z_ps.tile([DH + 1, QCH], f32, tag="z", name="zb")
                    zc = z_ps.tile([DH + 1, QCH], f32, tag="z", name="zc")
                    pend = None
                    for ki in range(nk):
                        vs = max(0, P * ki - q0)
                        sa = sc_ps.tile([P, QCH], f32, tag="S", name="sa")
                        sb = sc_ps.tile([P, QCH], f32, tag="S", name="sb")
                        sc = sc_ps.tile([P, QCH], f32, tag="S", name="sc")
                        nc.tensor.matmul(
                            sa[:, vs:], lhsT=KT2[0:DH, ki * P:(ki + 1) * P],
                            rhs=QT2[0:DH, q0 + vs:q0 + QCH],
                            start=True, stop=True)
                        nc.tensor.matmul(
                            sb[:, vs:], lhsT=KT2[DH:P, ki * P:(ki + 1) * P],
                            rhs=QT2[DH:P, q0 + vs:q0 + QCH],
                            start=True, stop=True)
                        nc.tensor.matmul(
                            sc[:, vs:], lhsT=KTs[:, ki * P:(ki + 1) * P],
                            rhs=QTs[:, q0 + vs:q0 + QCH],
                            start=True, stop=True)
                        pta = pt_pool.tile([P, QCH], bf16, tag="pta",
                                           name="pta")
                        nc.scalar.activation(pta[:, vs:], sa[:, vs:], Exp)
                        # head-1 exp alternates ScalarE/VectorE to balance
                        if ki % 2 == 0:
                            ptb = pt_pool.tile([P, QCH], bf16, tag="ptb",
                                               name="ptb")
                            nc.scalar.activation(ptb[:, vs:], sb[:, vs:], Exp)
                            ptb_bf = ptb[:]
                        else:
                            ptb = pt_pool.tile([P, QCH], i16, tag="ptbi",
                                               name="ptbi")
                            nc.vector.tensor_scalar(
                                ptb[:, vs:], sb[:, vs:], _AEXP, _BEXP,
                                mult, add)
                            ptb_bf = ptb[:].bitcast(bf16)
                        ptc = pt_pool.tile([P, QCH], i16, tag="ptc",
                                           name="ptc")
                        nc.vector.tensor_scalar(
                            ptc[:, vs:], sc[:, vs:], _AEXP, _BEXP, mult, add)
                        ptc_bf = ptc[:].bitcast(bf16)
                        if ki >= KPW * qs:  # diagonal tile: causal mask
                            for blk in (pta[:, vs:vs + P],
                                        ptb_bf[:, vs:vs + P],
                                        ptc_bf[:, vs:vs + P]):
                                nc.gpsimd.tensor_tensor(
                                    blk, blk, tri_sb[:], mult)
                        if ki == 0:
                            def emit_avs(kj, vj, a, b_, c_,
                                         za=za, zb=zb, zc=zc, nk=nk):
                                for h, zt, pt_ in ((0, za, a), (1, zb, b_),
                                                   (2, zc, c_)):
                                    nc.tensor.matmul(
                                        zt[:, vj:], lhsT=V_sb[:, h, kj, :],
                                        rhs=pt_[:, vj:],
                                        start=(kj == 0), stop=(kj == nk - 1))
                        if pend is not None:
                            emit_avs(*pend)
                            pop_fills()
                        pend = (ki, vs, pta, ptb_bf, ptc_bf)
                    emit_avs(*pend)
                    pop_fills()

                    # ---- stage1 inline: extract row sums; evacuate z ----
                    rro = []
                    for zt, nm in ((za, "a"), (zb, "b"), (zc, "c")):
                        rr = nrm.tile([1, QCH], bf16, tag=f"rr{nm}",
                                      name=f"rr{nm}")
                        nc.vector.tensor_copy(rr[:], zt[DH:DH + 1, :])
                        rro.append(rr)
                    rra, rrb_, rrc = rro
                    zsb2 = nrm.tile([P, QCH], bf16, tag="zsb2", name="zsb2")
                    nc.scalar.copy(zsb2[0:DH, :], za[0:DH, :])
                    tmpb = nrm.tile([DH, QCH], bf16, tag="tmpb", name="tmpb")
                    nc.vector.tensor_copy(tmpb[:], zb[0:DH, :])
                    nc.sync.dma_start(zsb2[DH:P, :], tmpb[:])
                    zsbc = nrm.tile([DH, QCH], bf16, tag="zsbc", name="zsbc")
                    nc.scalar.copy(zsbc[:], zc[0:DH, :])

                    # ---- stage the deferred work as PE fillers ----
                    fills.extend(
                        stage2_thunks(qs, (rra, rrb_, rrc, zsb2, zsbc)))
                    if qs >= 1:
                        fills.extend(oproj_thunks(qs - 1))
                while fills:
                    fills.pop(0)()
                for th in oproj_thunks(n_ch - 1):
                    th()

    nc.compile()
    return nc


def _prep_inputs(inputs, seq_len, use_biases):
    x = np.asarray(inputs["normalized_resid_pre"], dtype=np.float32)
    WQ = np.asarray(inputs["W_Q"], dtype=np.float32)
    WK = np.asarray(inputs["W_K"], dtype=np.float32)
    WV = np.asarray(inputs["W_V"], dtype=np.float32)
    WO = np.asarray(inputs["W_O"], dtype=np.float32)

    tri = np.triu(np.ones((P, P), np.float32)).astype(_BF)  # keep j >= p
    idb = np.eye(P, dtype=np.float32).astype(_BF)
    onz = np.ones((1, DH), np.float32).astype(_BF)
    elo = np.zeros((1, P), np.float32)
    elo[0, :DH] = 1.0
    elo = elo.astype(_BF)
    ehi = np.zeros((1, P), np.float32)
    ehi[0, DH:] = 1.0
    ehi = ehi.astype(_BF)

    in_maps = []
    for c in range(NCORES):
        b, g = divmod(c, GROUPS)
        hs = slice(g * HPC, (g + 1) * HPC)
        wo_g = WO[hs]  # [3, 64, 768]
        m = {
            "xT": np.ascontiguousarray(x[b, :seq_len].T).astype(_BF),
            # W_Q pre-scaled by 1/sqrt(d_head)=1/8 so scores psum = s/8
            "wq": np.ascontiguousarray(
                (WQ[hs] / 8.0).transpose(1, 0, 2).reshape(DM, HPC * DH)
            ).astype(_BF),
            "wk": np.ascontiguousarray(
                WK[hs].transpose(1, 0, 2).reshape(DM, HPC * DH)).astype(_BF),
            "wv": np.ascontiguousarray(
                WV[hs].transpose(1, 0, 2).reshape(DM, HPC * DH)).astype(_BF),
            "wo_p": np.ascontiguousarray(
                wo_g[0:2].reshape(2 * DH, DM)).astype(_BF),
            "wo_s": np.ascontiguousarray(wo_g[2]).astype(_BF),
            "trimask": tri,
            "ident_b": idb,
            "ones_z": onz,
            "e_lo": elo,
            "e_hi": ehi,
        }
        if use_biases:
            bq = np.asarray(inputs["b_Q"], np.float32)[hs] / 8.0
            bk = np.asarray(inputs["b_K"], np.float32)[hs]
            bv = np.asarray(inputs["b_V"], np.float32)[hs]
            # pair layout: [128, 3] = heads {0,1} stacked, cols q/k/v
            m["bqkv_p"] = np.stack(
                [np.concatenate([bq[0], bq[1]]),
                 np.concatenate([bk[0], bk[1]]),
                 np.concatenate([bv[0], bv[1]])], axis=1)
            m["bqkv_s"] = np.stack([bq[2], bk[2], bv[2]], axis=1)
        in_maps.append(m)
    return in_maps


TRACE = False          # test.py can flip this to get exec_time_ns
last_result = None     # BassKernelResults of the most recent run


def kernel(seq_len=S, **inputs):
    global last_result
    from concourse.bass_utils import run_bass_kernel_spmd

    use_biases = any(
        np.any(np.asarray(inputs[k]) != 0) for k in ("b_Q", "b_K", "b_V"))

    key = (seq_len, use_biases)
    if key not in _cache:
        _cache[key] = _build(seq_len, use_biases)
    nc = _cache[key]

    in_maps = _prep_inputs(inputs, seq_len, use_biases)
    res = run_bass_kernel_spmd(nc, in_maps, core_ids=list(range(NCORES)),
                               trace=TRACE)
    last_result = res

    b_O = np.asarray(inputs["b_O"], dtype=np.float32)
    out = np.zeros((B, seq_len, DM), np.float32)
    for c in range(NCORES):
        b = c // GROUPS
        out[b] += np.asarray(res.results[c]["out"], dtype=np.float32)
    out += b_O[None, None, :]
    return out


# revision 25
# speedup vs baseline: 1.2740x; 1.0835x over previous
"""Causal multi-head attention on 8 Trainium2 NeuronCores.

Problem: B=2, S=4096, D_MODEL=768, H=12, D_HEAD=64, fp32 I/O.

Sharding: (batch, head-group) -> core.  Cores 0-3 take batch 0, cores 4-7
take batch 1; each core computes 3 of the 12 heads for its batch and emits a
partial output [S, D_MODEL] (its heads' contribution to the W_O contraction).
The host sums the 4 partials per batch and adds b_O.

Per-core device program (matmul compute in bf16, fp32 PSUM accumulation):
  1. QT/KT[z, t] = W.T @ xT; heads 0,1 packed on partition halves (0-63 /
     64-127) so their scores matmuls run concurrently in different PE row
     groups; head 2 separate.  W_Q is pre-scaled by 1/8 host-side.  VT
     computed per head-pair/single, then PE-transposed to V[t, z] with a ones
     column appended (softmax row sums ride along the AV matmul).
  2. Flash attention over 512-wide query windows, one interleaved loop per
     window covering all three heads per k-tile.  The three exps are split
     across engines so no engine gates the PE: head 0 on ScalarE (ACT Exp),
     heads 1,2 on VectorE via a fused Schraudolph fast-exp (tensor_scalar
     mult+add with int16 output whose bits ARE the bf16 encoding of e^x;
     ~2% rel err, fine at the 2e-2 gate).  Causal diag masks run on GpSimd.
     Score matmuls are emitted one k-tile ahead of the AV matmuls so the PE
     never waits on an exp.  PSUM: 4-slot score ring + 3 z slots + 1 fill
     slot = 8 banks exactly.
  3. Softmax normalization: row sums (row 64 of each z psum) are copied out,
     inverted with the 1-op approx reciprocal, broadcast across partitions by
     a tiny rank-2/rank-1 f32r matmul, and applied with one multiply per
     head-pair.  z for heads 0,1 is restacked onto partitions 0:63 / 64:127
     (SBUF->SBUF DMA for the high half) so the output projection contracts
     both heads in a single C=128 matmul.
  4. Output projection per 128-row tile: one C=128 matmul (heads 0,1) plus
     one C=64 accumulating matmul (head 2) per 384-wide half; psum is copied
     out on ScalarE/VectorE alternately and DMA'd.  Normalization broadcasts
     and O-proj tiles are deferred into a fill list drained one-two per
     k-tile to keep the PE dense (a >3.4us PE stall re-throttles the PE
     clock from 2.4 to 1.2 GHz).
"""

import numpy as np
import ml_dtypes

B, S, DM, H, DH = 2, 4096, 768, 12, 64
NCORES = 8
GROUPS = 4                  # head-groups per batch
HPC = H // GROUPS           # heads per core = 3
P = 128
QCH = 512                   # psum bank width (fp32)

_BF = ml_dtypes.bfloat16

# Schraudolph fast-exp constants: int16 value = round(x*AEXP + BEXP) is the
# bf16 bit pattern of e^x (C=367400 tuned for min max-rel-err, ~2% RMS).
_AEXP = (2.0 ** 23 / np.log(2.0)) / 65536.0
_BEXP = (127.0 * 2.0 ** 23 - 367400.0) / 65536.0

_cache = {}


def _build(seq_len, use_biases):
    import concourse.bacc as bacc
    import concourse.mybir as mybir
    import concourse.tile as tile

    f32 = mybir.dt.float32
    f32r = mybir.dt.float32r
    bf16 = mybir.dt.bfloat16
    i16 = mybir.dt.int16
    Exp = mybir.ActivationFunctionType.Exp
    mult = mybir.AluOpType.mult
    add = mybir.AluOpType.add

    SQ = seq_len
    n_kt = SQ // P               # k tiles
    n_ch = SQ // QCH             # 512-wide chunks
    DSL = DM // P                # contraction slices for the projections
    KPW = QCH // P               # k tiles per query window

    nc = bacc.Bacc(None, target_bir_lowering=False)

    xT = nc.declare_dram_parameter("xT", [DM, SQ], bf16, isOutput=False)
    wq = nc.declare_dram_parameter("wq", [DM, HPC * DH], bf16, isOutput=False)
    wk = nc.declare_dram_parameter("wk", [DM, HPC * DH], bf16, isOutput=False)
    wv = nc.declare_dram_parameter("wv", [DM, HPC * DH], bf16, isOutput=False)
    wo_p = nc.declare_dram_parameter("wo_p", [2 * DH, DM], bf16, isOutput=False)
    wo_s = nc.declare_dram_parameter("wo_s", [DH, DM], bf16, isOutput=False)
    trimask = nc.declare_dram_parameter("trimask", [P, P], bf16, isOutput=False)
    ident_b = nc.declare_dram_parameter("ident_b", [P, P], bf16, isOutput=False)
    ones_z = nc.declare_dram_parameter("ones_z", [1, DH], bf16, isOutput=False)
    e_lo = nc.declare_dram_parameter("e_lo", [1, P], bf16, isOutput=False)
    e_hi = nc.declare_dram_parameter("e_hi", [1, P], bf16, isOutput=False)
    if use_biases:
        bqkv_p = nc.declare_dram_parameter("bqkv_p", [P, 3], f32, isOutput=False)
        bqkv_s = nc.declare_dram_parameter("bqkv_s", [DH, 3], f32, isOutput=False)
    out = nc.declare_dram_parameter("out", [SQ, DM], f32, isOutput=True)

    with tile.TileContext(nc) as tc:
        with (
            tc.tile_pool(name="singles", bufs=1) as singles,
            tc.tile_pool(name="persist", bufs=1) as persist,
            tc.tile_pool(name="nrm", bufs=2) as nrm,
        ):
            # ---- constants / weights ----
            w_sb = {}
            for name, drm in (("q", wq), ("k", wk), ("v", wv)):
                t = singles.tile([P, DSL, HPC * DH], bf16, tag=f"w{name}")
                nc.sync.dma_start(t[:], drm.rearrange("(o p) c -> p o c", p=P))
                w_sb[name] = t
            wop_sb = singles.tile([2 * DH, DM], bf16)
            nc.sync.dma_start(wop_sb[:], wo_p[:])
            wos_sb = singles.tile([DH, DM], bf16)
            nc.sync.dma_start(wos_sb[:], wo_s[:])
            tri_sb = singles.tile([P, P], bf16)
            nc.sync.dma_start(tri_sb[:], trimask[:])
            idb_sb = singles.tile([P, P], bf16)
            nc.sync.dma_start(idb_sb[:], ident_b[:])
            ones_sb = singles.tile([1, DH], bf16)
            nc.sync.dma_start(ones_sb[:], ones_z[:])
            elo_sb = singles.tile([1, P], bf16, tag="elo")
            nc.sync.dma_start(elo_sb[:], e_lo[:])
            ehi_sb = singles.tile([1, P], bf16, tag="ehi")
            nc.sync.dma_start(ehi_sb[:], e_hi[:])
            bias_p = bias_s = None
            if use_biases:
                bias_p = singles.tile([P, 3], f32, tag="bp")
                nc.sync.dma_start(bias_p[:], bqkv_p[:])
                bias_s = singles.tile([DH, 3], f32, tag="bs")
                nc.sync.dma_start(bias_s[:], bqkv_s[:])

            # ---- persistent activations ----
            QT2 = persist.tile([P, SQ], bf16, tag="QT2")   # heads 0,1 stacked
            KT2 = persist.tile([P, SQ], bf16, tag="KT2")
            QTs = persist.tile([DH, SQ], bf16, tag="QTs")  # head 2
            KTs = persist.tile([DH, SQ], bf16, tag="KTs")
            V_sb = persist.tile([P, HPC, n_kt, DH + 1], bf16, tag="V")
            Zn2 = persist.tile([P, SQ], bf16, tag="Zn2")   # normalized z h0|h1
            Zns = persist.tile([DH, SQ], bf16, tag="Zns")  # normalized z h2

            # ================= QKV projections =================
            with (
                tc.tile_pool(name="xT_pool", bufs=1) as xT_pool,
                tc.tile_pool(name="qkv_ps", bufs=3, space="PSUM") as qkv_ps,
                tc.tile_pool(name="vt_ps", bufs=3, space="PSUM") as vt_ps,
                tc.tile_pool(name="vt_sb", bufs=1) as vt_pool,
            ):
                # one chunked DMA per 512-col slab: single queue-issue each,
                # fine-grained enough that proj chunk c starts early.
                xT_sb = xT_pool.tile([P, DSL, SQ], bf16)
                xT_r = xT.rearrange("(o p) c -> p o c", p=P)
                for c in range(n_ch):
                    nc.sync.dma_start(
                        xT_sb[:, :, c * QCH:(c + 1) * QCH],
                        xT_r[:, :, c * QCH:(c + 1) * QCH])
                # HAM warm-up: keep the PE MAC-busy while xT streams in, so
                # the first real matmuls run at 2.4 GHz instead of 1.2.
                wup = vt_ps.tile([P, P], f32, tag="vtp", name="wup")
                for _ in range(90):
                    nc.tensor.matmul(wup[:], lhsT=idb_sb[:], rhs=idb_sb[:],
                                     start=True, stop=True)

                def proj(tname, w_cols, dst, bias, c, evac):
                    m = w_cols.stop - w_cols.start
                    ps = qkv_ps.tile([P, QCH], f32, tag="proj",
                                     name="proj_ps")[:m]
                    for o in range(DSL):
                        nc.tensor.matmul(
                            ps[:],
                            lhsT=w_sb[tname][:, o, w_cols],
                            rhs=xT_sb[:, o, c * QCH:(c + 1) * QCH],
                            start=(o == 0), stop=(o == DSL - 1),
                        )
                    if bias is not None:
                        if evac == 0:
                            nc.scalar.add(dst, ps[:], bias)
                        else:
                            nc.vector.tensor_scalar_add(dst, ps[:], bias)
                    else:
                        if evac == 0:
                            nc.scalar.copy(dst, ps[:])
                        else:
                            nc.vector.tensor_copy(dst, ps[:])

                for tname, d2, ds, bi in (("q", QT2, QTs, 0),
                                          ("k", KT2, KTs, 1)):
                    for c in range(n_ch):
                        proj(tname, slice(0, P),
                             d2[:, c * QCH:(c + 1) * QCH],
                             bias_p[:, bi:bi + 1] if use_biases else None,
                             c, c % 2)
                    for c in range(n_ch):
                        proj(tname, slice(P, P + DH),
                             ds[:, c * QCH:(c + 1) * QCH],
                             bias_s[:, bi:bi + 1] if use_biases else None,
                             c, c % 2)
                # V: pair pass (M=128) + single pass, then batched transposes
                vt2 = vt_pool.tile([P, SQ], bf16, tag="vt2")
                for c in range(n_ch):
                    proj("v", slice(0, P), vt2[:, c * QCH:(c + 1) * QCH],
                         bias_p[:, 2:3] if use_biases else None, c, c % 2)
                vts = vt_pool.tile([DH, SQ], bf16, tag="vts")
                for c in range(n_ch):
                    proj("v", slice(P, P + DH),
                         vts[:, c * QCH:(c + 1) * QCH],
                         bias_s[:, 2:3] if use_biases else None, c, c % 2)
                # paired transposes: 2 k-tiles per psum tile / per DVE copy
                for kt in range(0, n_kt, 2):
                    for h, srcs, idsl in (
                            (0, vt2[0:DH], idb_sb[:DH, :DH]),
                            (1, vt2[DH:P], idb_sb[DH:P, DH:P]),
                            (2, vts[:], idb_sb[:DH, :DH])):
                        vp = vt_ps.tile([P, 2 * DH], bf16, tag="vtp",
                                        name="vp")
                        nc.tensor.transpose(
                            vp[:, 0:DH], srcs[:, kt * P:(kt + 1) * P], idsl)
                        nc.tensor.transpose(
                            vp[:, DH:2 * DH],
                            srcs[:, (kt + 1) * P:(kt + 2) * P], idsl)
                        nc.vector.tensor_copy(
                            V_sb[:, h, kt:kt + 2, 0:DH],
                            vp[:].rearrange("p (j z) -> p j z", j=2))
                nc.vector.memset(V_sb[:, :, :, DH:DH + 1], 1.0)

            # ===== flash: interleaved heads, one 512-wide window loop ====
            with (
                tc.tile_pool(name="sc_ps", bufs=4, space="PSUM") as sc_ps,
                tc.tile_pool(name="z_ps", bufs=3, space="PSUM") as z_ps,
                tc.tile_pool(name="fill_ps", bufs=1, space="PSUM") as fill_ps,
                tc.tile_pool(name="pt_sb", bufs=3) as pt_pool,
                tc.tile_pool(name="o_sb", bufs=6) as o_pool,
            ):
                HD = DM // 2
                fills = []

                def pop_fills(k=2):
                    for _ in range(k):
                        if fills:
                            fills.pop(0)()

                def stage2_thunks(qs, st):
                    """Broadcast row sums, invert post-broadcast, normalize."""
                    rra, rrb_, rrc, zsb2, zsbc = st
                    q0 = qs * QCH

                    def th_pair():
                        rb = fill_ps.tile([P, QCH], f32, tag="fill",
                                          name="rb2")
                        nc.tensor.matmul(rb[:], lhsT=elo_sb[:], rhs=rra[:],
                                         start=True, stop=False)
                        nc.tensor.matmul(rb[:], lhsT=ehi_sb[:], rhs=rrb_[:],
                                         start=False, stop=True)
                        rq = nrm.tile([P, QCH], f32, tag="rq2", name="rq2")
                        nc.vector.reciprocal_approx_fast(rq[:], rb[:])
                        nc.vector.tensor_tensor(
                            Zn2[:, q0:q0 + QCH], zsb2[:], rq[:], mult)

                    def th_single():
                        rb = fill_ps.tile([P, QCH], f32, tag="fill",
                                          name="rbc")[:DH]
                        nc.tensor.matmul(rb[:], lhsT=ones_sb[:], rhs=rrc[:],
                                         start=True, stop=True)
                        rq = nrm.tile([DH, QCH], f32, tag="rqs", name="rqs")
                        nc.vector.reciprocal_approx_fast(rq[:], rb[:])
                        nc.vector.tensor_tensor(
                            Zns[:, q0:q0 + QCH], zsbc[:], rq[:], mult)

                    return [th_pair, th_single]

                osb_rr = [0]  # 5:3 scalar/vector rotation for osb copies
                _OSB_PAT = (0, 1, 0, 0, 1, 0, 0, 1)

                def oproj_thunks(w):
                    """O-proj for window w as per-half-tile thunks."""
                    thunks = []
                    for tt in range(w * KPW, (w + 1) * KPW):
                        osb = o_pool.tile([P, DM], f32, tag="osb", name="osb")

                        def th(tt=tt, osb=osb, half=0):
                            po = fill_ps.tile([P, QCH], f32, tag="fill",
                                              name="po")[:, :HD]
                            nc.tensor.matmul(
                                po[:],
                                lhsT=Zn2[:, tt * P:(tt + 1) * P],
                                rhs=wop_sb[:, half * HD:(half + 1) * HD],
                                start=True, stop=False)
                            nc.tensor.matmul(
                                po[:],
                                lhsT=Zns[:, tt * P:(tt + 1) * P],
                                rhs=wos_sb[:, half * HD:(half + 1) * HD],
                                start=False, stop=True)
                            eng = _OSB_PAT[osb_rr[0] % 8]
                            osb_rr[0] += 1
                            if eng == 0:
                                nc.scalar.copy(
                                    osb[:, half * HD:(half + 1) * HD], po[:])
                            else:
                                nc.vector.tensor_copy(
                                    osb[:, half * HD:(half + 1) * HD], po[:])
                            if half == 1:
                                nc.sync.dma_start(
                                    out[tt * P:(tt + 1) * P, :], osb[:])

                        thunks.append(th)
                        thunks.append(lambda tt=tt, osb=osb, th=th: th(tt, osb, 1))
                    return thunks

                for qs in range(n_ch):
                    q0 = qs * QCH
                    nk = KPW * qs + KPW
                    za = z_ps.tile([DH + 1, QCH], f32, tag="z", name="za")
                    zb = z_ps.tile([DH + 1, QCH], f32, tag="z", name="zb")
                    zc = z_ps.tile([DH + 1, QCH], f32, tag="z", name="zc")
                    pend = []
                    for ki in range(nk):
                        vs = max(0, P * ki - q0)
                        sa = sc_ps.tile([P, QCH], f32, tag="S", name="sa")
                        sb = sc_ps.tile([P, QCH], f32, tag="S", name="sb")
                        sc = sc_ps.tile([P, QCH], f32, tag="S", name="sc")
                        nc.tensor.matmul(
                            sa[:, vs:], lhsT=KT2[0:DH, ki * P:(ki + 1) * P],
                            rhs=QT2[0:DH, q0 + vs:q0 + QCH],
                            start=True, stop=True)
                        nc.tensor.matmul(
                            sb[:, vs:], lhsT=KT2[DH:P, ki * P:(ki + 1) * P],
                            rhs=QT2[DH:P, q0 + vs:q0 + QCH],
                            start=True, stop=True)
                        nc.tensor.matmul(
                            sc[:, vs:], lhsT=KTs[:, ki * P:(ki + 1) * P],
                            rhs=QTs[:, q0 + vs:q0 + QCH],
                            start=True, stop=True)
                        pta = pt_pool.tile([P, QCH], bf16, tag="pta",
                                           name="pta")
                        nc.scalar.activation(pta[:, vs:], sa[:, vs:], Exp)
                        # head-1 exp alternates ScalarE/VectorE to balance
                        if ki % 2 == 0:
                            ptb = pt_pool.tile([P, QCH], bf16, tag="ptb",
                                               name="ptb")
                            nc.scalar.activation(ptb[:, vs:], sb[:, vs:], Exp)
                            ptb_bf = ptb[:]
                        else:
                            ptb = pt_pool.tile([P, QCH], i16, tag="ptbi",
                                               name="ptbi")
                            nc.vector.tensor_scalar(
                                ptb[:, vs:], sb[:, vs:], _AEXP, _BEXP,
                                mult, add)
                            ptb_bf = ptb[:].bitcast(bf16)
                        ptc = pt_pool.tile([P, QCH], i16, tag="ptc",
                                           name="ptc")
                        nc.vector.tensor_scalar(
                            ptc[:, vs:], sc[:, vs:], _AEXP, _BEXP, mult, add)
                        ptc_bf = ptc[:].bitcast(bf16)
                        if ki >= KPW * qs:  # diagonal tile: causal mask
                            for blk in (pta[:, vs:vs + P],
                                        ptb_bf[:, vs:vs + P],
                                        ptc_bf[:, vs:vs + P]):
                                nc.gpsimd.tensor_tensor(
                                    blk, blk, tri_sb[:], mult)
                        if ki == 0:
                            def emit_avs(kj, vj, a, b_, c_,
                                         za=za, zb=zb, zc=zc, nk=nk):
                                for h, zt, pt_ in ((0, za, a), (1, zb, b_),
                                                   (2, zc, c_)):
                                    nc.tensor.matmul(
                                        zt[:, vj:], lhsT=V_sb[:, h, kj, :],
                                        rhs=pt_[:, vj:],
                                        start=(kj == 0), stop=(kj == nk - 1))
                        pend.append((ki, vs, pta, ptb_bf, ptc_bf))
                        # AV matmuls lag the scores by 2 k-tiles so the
                        # cross-engine exp latency never stalls the PE.
                        if len(pend) > 2:
                            emit_avs(*pend.pop(0))
                            pop_fills()
                    while pend:
                        emit_avs(*pend.pop(0))
                        pop_fills()

                    # ---- stage1 inline: extract row sums; evacuate z ----
                    rro = []
                    for zt, nm in ((za, "a"), (zb, "b"), (zc, "c")):
                        rr = nrm.tile([1, QCH], bf16, tag=f"rr{nm}",
                                      name=f"rr{nm}")
                        nc.vector.tensor_copy(rr[:], zt[DH:DH + 1, :])
                        rro.append(rr)
                    rra, rrb_, rrc = rro
                    zsb2 = nrm.tile([P, QCH], bf16, tag="zsb2", name="zsb2")
                    nc.scalar.copy(zsb2[0:DH, :], za[0:DH, :])
                    tmpb = nrm.tile([DH, QCH], bf16, tag="tmpb", name="tmpb")
                    nc.vector.tensor_copy(tmpb[:], zb[0:DH, :])
                    nc.sync.dma_start(zsb2[DH:P, :], tmpb[:])
                    zsbc = nrm.tile([DH, QCH], bf16, tag="zsbc", name="zsbc")
                    nc.scalar.copy(zsbc[:], zc[0:DH, :])

                    # ---- stage the deferred work as PE fillers ----
                    fills.extend(
                        stage2_thunks(qs, (rra, rrb_, rrc, zsb2, zsbc)))
                    if 1 <= qs < n_ch - 1:
                        fills.extend(oproj_thunks(qs - 1))
                    if qs == n_ch - 2:
                        # release this window's o-proj into the last window
                        # too, so only window n_ch-1's work drains at the end
                        fills.extend(oproj_thunks(qs))
                while fills:
                    fills.pop(0)()
                for th in oproj_thunks(n_ch - 1):
                    th()

    nc.compile()
    return nc


def _prep_inputs(inputs, seq_len, use_biases):
    x = np.asarray(inputs["normalized_resid_pre"], dtype=np.float32)
    WQ = np.asarray(inputs["W_Q"], dtype=np.float32)
    WK = np.asarray(inputs["W_K"], dtype=np.float32)
    WV = np.asarray(inputs["W_V"], dtype=np.float32)
    WO = np.asarray(inputs["W_O"], dtype=np.float32)

    tri = np.triu(np.ones((P, P), np.float32)).astype(_BF)  # keep j >= p
    idb = np.eye(P, dtype=np.float32).astype(_BF)
    onz = np.ones((1, DH), np.float32).astype(_BF)
    elo = np.zeros((1, P), np.float32)
    elo[0, :DH] = 1.0
    elo = elo.astype(_BF)
    ehi = np.zeros((1, P), np.float32)
    ehi[0, DH:] = 1.0
    ehi = ehi.astype(_BF)

    in_maps = []
    for c in range(NCORES):
        b, g = divmod(c, GROUPS)
        hs = slice(g * HPC, (g + 1) * HPC)
        wo_g = WO[hs]  # [3, 64, 768]
        m = {
            "xT": np.ascontiguousarray(x[b, :seq_len].T).astype(_BF),
            # W_Q pre-scaled by 1/sqrt(d_head)=1/8 so scores psum = s/8
            "wq": np.ascontiguousarray(
                (WQ[hs] / 8.0).transpose(1, 0, 2).reshape(DM, HPC * DH)
            ).astype(_BF),
            "wk": np.ascontiguousarray(
                WK[hs].transpose(1, 0, 2).reshape(DM, HPC * DH)).astype(_BF),
            "wv": np.ascontiguousarray(
                WV[hs].transpose(1, 0, 2).reshape(DM, HPC * DH)).astype(_BF),
            "wo_p": np.ascontiguousarray(
                wo_g[0:2].reshape(2 * DH, DM)).astype(_BF),
            "wo_s": np.ascontiguousarray(wo_g[2]).astype(_BF),
            "trimask": tri,
            "ident_b": idb,
            "ones_z": onz,
            "e_lo": elo,
            "e_hi": ehi,
        }
        if use_biases:
            bq = np.asarray(inputs["b_Q"], np.float32)[hs] / 8.0
            bk = np.asarray(inputs["b_K"], np.float32)[hs]
            bv = np.asarray(inputs["b_V"], np.float32)[hs]
            # pair layout: [128, 3] = heads {0,1} stacked, cols q/k/v
            m["bqkv_p"] = np.stack(
                [np.concatenate([bq[0], bq[1]]),
                 np.concatenate([bk[0], bk[1]]),
                 np.concatenate([bv[0], bv[1]])], axis=1)
            m["bqkv_s"] = np.stack([bq[2], bk[2], bv[2]], axis=1)
        in_maps.append(m)
    return in_maps


TRACE = False          # test.py can flip this to get exec_time_ns
last_result = None     # BassKernelResults of the most recent run


def kernel(seq_len=S, **inputs):
    global last_result
    from concourse.bass_utils import run_bass_kernel_spmd

    use_biases = any(
        np.any(np.asarray(inputs[k]) != 0) for k in ("b_Q", "b_K", "b_V"))

    key = (seq_len, use_biases)
    if key not in _cache:
        _cache[key] = _build(seq_len, use_biases)
    nc = _cache[key]

    in_maps = _prep_inputs(inputs, seq_len, use_biases)
    res = run_bass_kernel_spmd(nc, in_maps, core_ids=list(range(NCORES)),
                               trace=TRACE)
    last_result = res

    b_O = np.asarray(inputs["b_O"], dtype=np.float32)
    out = np.zeros((B, seq_len, DM), np.float32)
    for c in range(NCORES):
        b = c // GROUPS
        out[b] += np.asarray(res.results[c]["out"], dtype=np.float32)
    out += b_O[None, None, :]
    return out


# revision 26
# speedup vs baseline: 1.3231x; 1.0385x over previous
"""Causal multi-head attention on 8 Trainium2 NeuronCores.

Problem: B=2, S=4096, D_MODEL=768, H=12, D_HEAD=64, fp32 I/O.

Sharding: (batch, head-group) -> core.  Cores 0-3 take batch 0, cores 4-7
take batch 1; each core computes 3 of the 12 heads for its batch and emits a
partial output [S, D_MODEL] (its heads' contribution to the W_O contraction).
The host sums the 4 partials per batch and adds b_O.

Per-core device program (matmul compute in bf16, fp32 PSUM accumulation):
  1. QT/KT[z, t] = W.T @ xT; heads 0,1 packed on partition halves (0-63 /
     64-127) so their scores matmuls run concurrently in different PE row
     groups; head 2 separate.  W_Q is pre-scaled by 1/8 host-side.  VT
     computed per head-pair/single, then PE-transposed to V[t, z] with a ones
     column appended (softmax row sums ride along the AV matmul).
  2. Flash attention over 512-wide query windows, one interleaved loop per
     window covering all three heads per k-tile.  The three exps are split
     across engines so no engine gates the PE: head 0 on ScalarE (ACT Exp),
     heads 1,2 on VectorE via a fused Schraudolph fast-exp (tensor_scalar
     mult+add with int16 output whose bits ARE the bf16 encoding of e^x;
     ~2% rel err, fine at the 2e-2 gate).  Causal diag masks run on GpSimd.
     Score matmuls are emitted one k-tile ahead of the AV matmuls so the PE
     never waits on an exp.  PSUM: 4-slot score ring + 3 z slots + 1 fill
     slot = 8 banks exactly.
  3. Softmax normalization: row sums (row 64 of each z psum) are copied out,
     inverted with the 1-op approx reciprocal, broadcast across partitions by
     a tiny rank-2/rank-1 f32r matmul, and applied with one multiply per
     head-pair.  z for heads 0,1 is restacked onto partitions 0:63 / 64:127
     (SBUF->SBUF DMA for the high half) so the output projection contracts
     both heads in a single C=128 matmul.
  4. Output projection per 128-row tile: one C=128 matmul (heads 0,1) plus
     one C=64 accumulating matmul (head 2) per 384-wide half; psum is copied
     out on ScalarE/VectorE alternately and DMA'd.  Normalization broadcasts
     and O-proj tiles are deferred into a fill list drained one-two per
     k-tile to keep the PE dense (a >3.4us PE stall re-throttles the PE
     clock from 2.4 to 1.2 GHz).
"""

import numpy as np
import ml_dtypes

B, S, DM, H, DH = 2, 4096, 768, 12, 64
NCORES = 8
GROUPS = 4                  # head-groups per batch
HPC = H // GROUPS           # heads per core = 3
P = 128
QCH = 512                   # psum bank width (fp32)

_BF = ml_dtypes.bfloat16

# Schraudolph fast-exp constants: int16 value = round(x*AEXP + BEXP) is the
# bf16 bit pattern of e^x (C=367400 tuned for min max-rel-err, ~2% RMS).
_AEXP = (2.0 ** 23 / np.log(2.0)) / 65536.0
_BEXP = (127.0 * 2.0 ** 23 - 367400.0) / 65536.0

_cache = {}


def _build(seq_len, use_biases):
    import concourse.bacc as bacc
    import concourse.mybir as mybir
    import concourse.tile as tile

    f32 = mybir.dt.float32
    f32r = mybir.dt.float32r
    bf16 = mybir.dt.bfloat16
    i16 = mybir.dt.int16
    Exp = mybir.ActivationFunctionType.Exp
    mult = mybir.AluOpType.mult
    add = mybir.AluOpType.add

    SQ = seq_len
    n_kt = SQ // P               # k tiles
    n_ch = SQ // QCH             # 512-wide chunks
    DSL = DM // P                # contraction slices for the projections
    KPW = QCH // P               # k tiles per query window

    nc = bacc.Bacc(None, target_bir_lowering=False)

    xT = nc.declare_dram_parameter("xT", [DM, SQ], bf16, isOutput=False)
    wq = nc.declare_dram_parameter("wq", [DM, HPC * DH], bf16, isOutput=False)
    wk = nc.declare_dram_parameter("wk", [DM, HPC * DH], bf16, isOutput=False)
    wv = nc.declare_dram_parameter("wv", [DM, HPC * DH], bf16, isOutput=False)
    wo_p = nc.declare_dram_parameter("wo_p", [2 * DH, DM], bf16, isOutput=False)
    wo_s = nc.declare_dram_parameter("wo_s", [DH, DM], bf16, isOutput=False)
    trimask = nc.declare_dram_parameter("trimask", [P, P], bf16, isOutput=False)
    ident_b = nc.declare_dram_parameter("ident_b", [P, P], bf16, isOutput=False)
    ones_z = nc.declare_dram_parameter("ones_z", [1, DH], bf16, isOutput=False)
    e_lo = nc.declare_dram_parameter("e_lo", [1, P], bf16, isOutput=False)
    e_hi = nc.declare_dram_parameter("e_hi", [1, P], bf16, isOutput=False)
    if use_biases:
        bqkv_p = nc.declare_dram_parameter("bqkv_p", [P, 3], f32, isOutput=False)
        bqkv_s = nc.declare_dram_parameter("bqkv_s", [DH, 3], f32, isOutput=False)
    out = nc.declare_dram_parameter("out", [SQ, DM], bf16, isOutput=True)

    with tile.TileContext(nc) as tc:
        with (
            tc.tile_pool(name="singles", bufs=1) as singles,
            tc.tile_pool(name="persist", bufs=1) as persist,
            tc.tile_pool(name="nrm", bufs=2) as nrm,
        ):
            # ---- constants / weights ----
            w_sb = {}
            for name, drm in (("q", wq), ("k", wk), ("v", wv)):
                t = singles.tile([P, DSL, HPC * DH], bf16, tag=f"w{name}")
                nc.sync.dma_start(t[:], drm.rearrange("(o p) c -> p o c", p=P))
                w_sb[name] = t
            wop_sb = singles.tile([2 * DH, DM], bf16)
            nc.sync.dma_start(wop_sb[:], wo_p[:])
            wos_sb = singles.tile([DH, DM], bf16)
            nc.sync.dma_start(wos_sb[:], wo_s[:])
            tri_sb = singles.tile([P, P], bf16)
            nc.sync.dma_start(tri_sb[:], trimask[:])
            idb_sb = singles.tile([P, P], bf16)
            nc.sync.dma_start(idb_sb[:], ident_b[:])
            ones_sb = singles.tile([1, DH], bf16)
            nc.sync.dma_start(ones_sb[:], ones_z[:])
            elo_sb = singles.tile([1, P], bf16, tag="elo")
            nc.sync.dma_start(elo_sb[:], e_lo[:])
            ehi_sb = singles.tile([1, P], bf16, tag="ehi")
            nc.sync.dma_start(ehi_sb[:], e_hi[:])
            bias_p = bias_s = None
            if use_biases:
                bias_p = singles.tile([P, 3], f32, tag="bp")
                nc.sync.dma_start(bias_p[:], bqkv_p[:])
                bias_s = singles.tile([DH, 3], f32, tag="bs")
                nc.sync.dma_start(bias_s[:], bqkv_s[:])

            # ---- persistent activations ----
            QT2 = persist.tile([P, SQ], bf16, tag="QT2")   # heads 0,1 stacked
            KT2 = persist.tile([P, SQ], bf16, tag="KT2")
            QTs = persist.tile([DH, SQ], bf16, tag="QTs")  # head 2
            KTs = persist.tile([DH, SQ], bf16, tag="KTs")
            V_sb = persist.tile([P, HPC, n_kt, DH + 1], bf16, tag="V")
            Zn2 = persist.tile([P, SQ], bf16, tag="Zn2")   # normalized z h0|h1
            Zns = persist.tile([DH, SQ], bf16, tag="Zns")  # normalized z h2

            # ================= QKV projections =================
            with (
                tc.tile_pool(name="xT_pool", bufs=1) as xT_pool,
                tc.tile_pool(name="qkv_ps", bufs=3, space="PSUM") as qkv_ps,
                tc.tile_pool(name="vt_ps", bufs=3, space="PSUM") as vt_ps,
                tc.tile_pool(name="vt_sb", bufs=1) as vt_pool,
            ):
                # one chunked DMA per 512-col slab: single queue-issue each,
                # fine-grained enough that proj chunk c starts early.
                xT_sb = xT_pool.tile([P, DSL, SQ], bf16)
                xT_r = xT.rearrange("(o p) c -> p o c", p=P)
                for c in range(n_ch):
                    nc.sync.dma_start(
                        xT_sb[:, :, c * QCH:(c + 1) * QCH],
                        xT_r[:, :, c * QCH:(c + 1) * QCH])
                # HAM warm-up: keep the PE MAC-busy while xT streams in, so
                # the first real matmuls run at 2.4 GHz instead of 1.2.
                wup = vt_ps.tile([P, P], f32, tag="vtp", name="wup")
                for _ in range(90):
                    nc.tensor.matmul(wup[:], lhsT=idb_sb[:], rhs=idb_sb[:],
                                     start=True, stop=True)

                def proj(tname, w_cols, dst, bias, c, evac):
                    m = w_cols.stop - w_cols.start
                    ps = qkv_ps.tile([P, QCH], f32, tag="proj",
                                     name="proj_ps")[:m]
                    for o in range(DSL):
                        nc.tensor.matmul(
                            ps[:],
                            lhsT=w_sb[tname][:, o, w_cols],
                            rhs=xT_sb[:, o, c * QCH:(c + 1) * QCH],
                            start=(o == 0), stop=(o == DSL - 1),
                        )
                    if bias is not None:
                        if evac == 0:
                            nc.scalar.add(dst, ps[:], bias)
                        else:
                            nc.vector.tensor_scalar_add(dst, ps[:], bias)
                    else:
                        if evac == 0:
                            nc.scalar.copy(dst, ps[:])
                        else:
                            nc.vector.tensor_copy(dst, ps[:])

                for tname, d2, ds, bi in (("q", QT2, QTs, 0),
                                          ("k", KT2, KTs, 1)):
                    for c in range(n_ch):
                        proj(tname, slice(0, P),
                             d2[:, c * QCH:(c + 1) * QCH],
                             bias_p[:, bi:bi + 1] if use_biases else None,
                             c, c % 2)
                    for c in range(n_ch):
                        proj(tname, slice(P, P + DH),
                             ds[:, c * QCH:(c + 1) * QCH],
                             bias_s[:, bi:bi + 1] if use_biases else None,
                             c, c % 2)
                # V: pair pass (M=128) + single pass, then batched transposes
                vt2 = vt_pool.tile([P, SQ], bf16, tag="vt2")
                for c in range(n_ch):
                    proj("v", slice(0, P), vt2[:, c * QCH:(c + 1) * QCH],
                         bias_p[:, 2:3] if use_biases else None, c, c % 2)
                vts = vt_pool.tile([DH, SQ], bf16, tag="vts")
                for c in range(n_ch):
                    proj("v", slice(P, P + DH),
                         vts[:, c * QCH:(c + 1) * QCH],
                         bias_s[:, 2:3] if use_biases else None, c, c % 2)
                # paired transposes: 2 k-tiles per psum tile / per DVE copy
                for kt in range(0, n_kt, 2):
                    for h, srcs, idsl in (
                            (0, vt2[0:DH], idb_sb[:DH, :DH]),
                            (1, vt2[DH:P], idb_sb[DH:P, DH:P]),
                            (2, vts[:], idb_sb[:DH, :DH])):
                        vp = vt_ps.tile([P, 2 * DH], bf16, tag="vtp",
                                        name="vp")
                        nc.tensor.transpose(
                            vp[:, 0:DH], srcs[:, kt * P:(kt + 1) * P], idsl)
                        nc.tensor.transpose(
                            vp[:, DH:2 * DH],
                            srcs[:, (kt + 1) * P:(kt + 2) * P], idsl)
                        nc.vector.tensor_copy(
                            V_sb[:, h, kt:kt + 2, 0:DH],
                            vp[:].rearrange("p (j z) -> p j z", j=2))
                nc.vector.memset(V_sb[:, :, :, DH:DH + 1], 1.0)

            # ===== flash: interleaved heads, one 512-wide window loop ====
            with (
                tc.tile_pool(name="sc_ps", bufs=4, space="PSUM") as sc_ps,
                tc.tile_pool(name="z_ps", bufs=3, space="PSUM") as z_ps,
                tc.tile_pool(name="fill_ps", bufs=1, space="PSUM") as fill_ps,
                tc.tile_pool(name="pt_sb", bufs=3) as pt_pool,
                tc.tile_pool(name="o_sb", bufs=6) as o_pool,
            ):
                HD = DM // 2
                fills = []

                def pop_fills(k=2):
                    for _ in range(k):
                        if fills:
                            fills.pop(0)()

                def stage2_thunks(qs, st):
                    """Broadcast row sums, invert post-broadcast, normalize."""
                    rra, rrb_, rrc, zsb2, zsbc = st
                    q0 = qs * QCH

                    def th_pair():
                        rb = fill_ps.tile([P, QCH], f32, tag="fill",
                                          name="rb2")
                        nc.tensor.matmul(rb[:], lhsT=elo_sb[:], rhs=rra[:],
                                         start=True, stop=False)
                        nc.tensor.matmul(rb[:], lhsT=ehi_sb[:], rhs=rrb_[:],
                                         start=False, stop=True)
                        rq = nrm.tile([P, QCH], f32, tag="rq2", name="rq2")
                        nc.vector.reciprocal_approx_fast(rq[:], rb[:])
                        nc.vector.tensor_tensor(
                            Zn2[:, q0:q0 + QCH], zsb2[:], rq[:], mult)

                    def th_single():
                        rb = fill_ps.tile([P, QCH], f32, tag="fill",
                                          name="rbc")[:DH]
                        nc.tensor.matmul(rb[:], lhsT=ones_sb[:], rhs=rrc[:],
                                         start=True, stop=True)
                        rq = nrm.tile([DH, QCH], f32, tag="rqs", name="rqs")
                        nc.vector.reciprocal_approx_fast(rq[:], rb[:])
                        nc.vector.tensor_tensor(
                            Zns[:, q0:q0 + QCH], zsbc[:], rq[:], mult)

                    return [th_pair, th_single]

                osb_rr = [0]  # 5:3 scalar/vector rotation for osb copies
                _OSB_PAT = (0, 1, 0, 0, 1, 0, 0, 1)

                def oproj_thunks(w):
                    """O-proj for window w as per-half-tile thunks."""
                    thunks = []
                    for tt in range(w * KPW, (w + 1) * KPW):
                        osb = o_pool.tile([P, DM], f32, tag="osb", name="osb")

                        def th(tt=tt, osb=osb, half=0):
                            po = fill_ps.tile([P, QCH], f32, tag="fill",
                                              name="po")[:, :HD]
                            nc.tensor.matmul(
                                po[:],
                                lhsT=Zn2[:, tt * P:(tt + 1) * P],
                                rhs=wop_sb[:, half * HD:(half + 1) * HD],
                                start=True, stop=False)
                            nc.tensor.matmul(
                                po[:],
                                lhsT=Zns[:, tt * P:(tt + 1) * P],
                                rhs=wos_sb[:, half * HD:(half + 1) * HD],
                                start=False, stop=True)
                            eng = _OSB_PAT[osb_rr[0] % 8]
                            osb_rr[0] += 1
                            if eng == 0:
                                nc.scalar.copy(
                                    osb[:, half * HD:(half + 1) * HD], po[:])
                            else:
                                nc.vector.tensor_copy(
                                    osb[:, half * HD:(half + 1) * HD], po[:])
                            if half == 1:
                                nc.sync.dma_start(
                                    out[tt * P:(tt + 1) * P, :], osb[:])

                        thunks.append(th)
                        thunks.append(lambda tt=tt, osb=osb, th=th: th(tt, osb, 1))
                    return thunks

                for qs in range(n_ch):
                    q0 = qs * QCH
                    nk = KPW * qs + KPW
                    za = z_ps.tile([DH + 1, QCH], f32, tag="z", name="za")
                    zb = z_ps.tile([DH + 1, QCH], f32, tag="z", name="zb")
                    zc = z_ps.tile([DH + 1, QCH], f32, tag="z", name="zc")
                    pend = []
                    for ki in range(nk):
                        vs = max(0, P * ki - q0)
                        sa = sc_ps.tile([P, QCH], f32, tag="S", name="sa")
                        sb = sc_ps.tile([P, QCH], f32, tag="S", name="sb")
                        sc = sc_ps.tile([P, QCH], f32, tag="S", name="sc")
                        nc.tensor.matmul(
                            sa[:, vs:], lhsT=KT2[0:DH, ki * P:(ki + 1) * P],
                            rhs=QT2[0:DH, q0 + vs:q0 + QCH],
                            start=True, stop=True)
                        nc.tensor.matmul(
                            sb[:, vs:], lhsT=KT2[DH:P, ki * P:(ki + 1) * P],
                            rhs=QT2[DH:P, q0 + vs:q0 + QCH],
                            start=True, stop=True)
                        nc.tensor.matmul(
                            sc[:, vs:], lhsT=KTs[:, ki * P:(ki + 1) * P],
                            rhs=QTs[:, q0 + vs:q0 + QCH],
                            start=True, stop=True)
                        pta = pt_pool.tile([P, QCH], bf16, tag="pta",
                                           name="pta")
                        nc.scalar.activation(pta[:, vs:], sa[:, vs:], Exp)
                        # head-1 exp alternates ScalarE/VectorE to balance
                        if ki % 2 == 0:
                            ptb = pt_pool.tile([P, QCH], bf16, tag="ptb",
                                               name="ptb")
                            nc.scalar.activation(ptb[:, vs:], sb[:, vs:], Exp)
                            ptb_bf = ptb[:]
                        else:
                            ptb = pt_pool.tile([P, QCH], i16, tag="ptbi",
                                               name="ptbi")
                            nc.vector.tensor_scalar(
                                ptb[:, vs:], sb[:, vs:], _AEXP, _BEXP,
                                mult, add)
                            ptb_bf = ptb[:].bitcast(bf16)
                        ptc = pt_pool.tile([P, QCH], i16, tag="ptc",
                                           name="ptc")
                        nc.vector.tensor_scalar(
                            ptc[:, vs:], sc[:, vs:], _AEXP, _BEXP, mult, add)
                        ptc_bf = ptc[:].bitcast(bf16)
                        if ki >= KPW * qs:  # diagonal tile: causal mask
                            for blk in (pta[:, vs:vs + P],
                                        ptb_bf[:, vs:vs + P],
                                        ptc_bf[:, vs:vs + P]):
                                nc.gpsimd.tensor_tensor(
                                    blk, blk, tri_sb[:], mult)
                        if ki == 0:
                            def emit_avs(kj, vj, a, b_, c_,
                                         za=za, zb=zb, zc=zc, nk=nk):
                                for h, zt, pt_ in ((0, za, a), (1, zb, b_),
                                                   (2, zc, c_)):
                                    nc.tensor.matmul(
                                        zt[:, vj:], lhsT=V_sb[:, h, kj, :],
                                        rhs=pt_[:, vj:],
                                        start=(kj == 0), stop=(kj == nk - 1))
                        pend.append((ki, vs, pta, ptb_bf, ptc_bf))
                        # AV matmuls lag the scores by 2 k-tiles so the
                        # cross-engine exp latency never stalls the PE.
                        if len(pend) > 2:
                            emit_avs(*pend.pop(0))
                            pop_fills()
                    while pend:
                        emit_avs(*pend.pop(0))
                        pop_fills()

                    # ---- stage1 inline: extract row sums; evacuate z ----
                    rro = []
                    for zt, nm in ((za, "a"), (zb, "b"), (zc, "c")):
                        rr = nrm.tile([1, QCH], bf16, tag=f"rr{nm}",
                                      name=f"rr{nm}")
                        nc.vector.tensor_copy(rr[:], zt[DH:DH + 1, :])
                        rro.append(rr)
                    rra, rrb_, rrc = rro
                    zsb2 = nrm.tile([P, QCH], bf16, tag="zsb2", name="zsb2")
                    nc.scalar.copy(zsb2[0:DH, :], za[0:DH, :])
                    tmpb = nrm.tile([DH, QCH], bf16, tag="tmpb", name="tmpb")
                    nc.vector.tensor_copy(tmpb[:], zb[0:DH, :])
                    nc.sync.dma_start(zsb2[DH:P, :], tmpb[:])
                    zsbc = nrm.tile([DH, QCH], bf16, tag="zsbc", name="zsbc")
                    nc.scalar.copy(zsbc[:], zc[0:DH, :])

                    # ---- stage the deferred work as PE fillers ----
                    fills.extend(
                        stage2_thunks(qs, (rra, rrb_, rrc, zsb2, zsbc)))
                    if 1 <= qs < n_ch - 1:
                        fills.extend(oproj_thunks(qs - 1))
                    if qs == n_ch - 2:
                        # release this window's o-proj into the last window
                        # too, so only window n_ch-1's work drains at the end
                        fills.extend(oproj_thunks(qs))
                while fills:
                    fills.pop(0)()
                for th in oproj_thunks(n_ch - 1):
                    th()

    nc.compile()
    return nc


def _prep_inputs(inputs, seq_len, use_biases):
    x = np.asarray(inputs["normalized_resid_pre"], dtype=np.float32)
    WQ = np.asarray(inputs["W_Q"], dtype=np.float32)
    WK = np.asarray(inputs["W_K"], dtype=np.float32)
    WV = np.asarray(inputs["W_V"], dtype=np.float32)
    WO = np.asarray(inputs["W_O"], dtype=np.float32)

    tri = np.triu(np.ones((P, P), np.float32)).astype(_BF)  # keep j >= p
    idb = np.eye(P, dtype=np.float32).astype(_BF)
    onz = np.ones((1, DH), np.float32).astype(_BF)
    elo = np.zeros((1, P), np.float32)
    elo[0, :DH] = 1.0
    elo = elo.astype(_BF)
    ehi = np.zeros((1, P), np.float32)
    ehi[0, DH:] = 1.0
    ehi = ehi.astype(_BF)

    in_maps = []
    for c in range(NCORES):
        b, g = divmod(c, GROUPS)
        hs = slice(g * HPC, (g + 1) * HPC)
        wo_g = WO[hs]  # [3, 64, 768]
        m = {
            "xT": np.ascontiguousarray(x[b, :seq_len].T).astype(_BF),
            # W_Q pre-scaled by 1/sqrt(d_head)=1/8 so scores psum = s/8
            "wq": np.ascontiguousarray(
                (WQ[hs] / 8.0).transpose(1, 0, 2).reshape(DM, HPC * DH)
            ).astype(_BF),
            "wk": np.ascontiguousarray(
                WK[hs].transpose(1, 0, 2).reshape(DM, HPC * DH)).astype(_BF),
            "wv": np.ascontiguousarray(
                WV[hs].transpose(1, 0, 2).reshape(DM, HPC * DH)).astype(_BF),
            "wo_p": np.ascontiguousarray(
                wo_g[0:2].reshape(2 * DH, DM)).astype(_BF),
            "wo_s": np.ascontiguousarray(wo_g[2]).astype(_BF),
            "trimask": tri,
            "ident_b": idb,
            "ones_z": onz,
            "e_lo": elo,
            "e_hi": ehi,
        }
        if use_biases:
            bq = np.asarray(inputs["b_Q"], np.float32)[hs] / 8.0
            bk = np.asarray(inputs["b_K"], np.float32)[hs]
            bv = np.asarray(inputs["b_V"], np.float32)[hs]
            # pair layout: [128, 3] = heads {0,1} stacked, cols q/k/v
            m["bqkv_p"] = np.stack(
                [np.concatenate([bq[0], bq[1]]),
                 np.concatenate([bk[0], bk[1]]),
                 np.concatenate([bv[0], bv[1]])], axis=1)
            m["bqkv_s"] = np.stack([bq[2], bk[2], bv[2]], axis=1)
        in_maps.append(m)
    return in_maps


TRACE = False          # test.py can flip this to get exec_time_ns
last_result = None     # BassKernelResults of the most recent run


def kernel(seq_len=S, **inputs):
    global last_result
    from concourse.bass_utils import run_bass_kernel_spmd

    use_biases = any(
        np.any(np.asarray(inputs[k]) != 0) for k in ("b_Q", "b_K", "b_V"))

    key = (seq_len, use_biases)
    if key not in _cache:
        _cache[key] = _build(seq_len, use_biases)
    nc = _cache[key]

    in_maps = _prep_inputs(inputs, seq_len, use_biases)
    res = run_bass_kernel_spmd(nc, in_maps, core_ids=list(range(NCORES)),
                               trace=TRACE)
    last_result = res

    b_O = np.asarray(inputs["b_O"], dtype=np.float32)
    out = np.zeros((B, seq_len, DM), np.float32)
    for c in range(NCORES):
        b = c // GROUPS
        out[b] += np.asarray(res.results[c]["out"], dtype=np.float32)
    out += b_O[None, None, :]
    return out
